# revision 27
# baseline (speedup 1.0000x reference)
"""Multi-head attention Trainium2 kernel (v3).

Full inputs -> shard over 8 NeuronCores (batch x head-group) -> full output.

Per core c: batch b = c // 2, head-group hg = c % 2 (8 of 16 heads).
Column-shard Wq/Wk/Wv, row-shard Wo; each core computes a partial output
projection for its batch; host sums the two partials per batch and adds bo.

v3: estimate-driven dense weave.  The softmax exp stream on ScalarE
(256 x ~1.15us) is the steady-state floor, so the schedule starts it as
early as possible (K0+Q0-quarter projections only) and then interleaves
all remaining PE work (K/Q projections, V-proj, attn@V, out-proj) into
the gaps between score matmul pairs, paced by a ns-level estimate of
each engine's backlog.  This keeps ScalarE ~always busy and the PE
stream dense (no HAM cold-throttle oscillation).

Layout (per core), same math as v2:
  - scores^T [k, q] with two heads of a pair packed on PE row halves
    (concurrent K=64 matmuls); exp on ScalarE; mask multiply on DVE.
  - V staged as [seq, 8 heads x 68] fp16 blocks: cols 0-63 = V, col 64 =
    ones (softmax denominators via the attn@V matmul), 65-67 pad.
  - normalization: reciprocal_approx_fast + gpsimd partition_broadcast
    + one DVE multiply into ctxT.
  - PSUM: shared proj/scores pool 2x[128,1024] (4 banks) + ctx
    3x[128,512] (3 banks) + out-proj 1x[128,512] (1 bank) = 8 banks.
"""

import os
import sys

for _p in ("/opt/trn_rl_repo", "/root/.axon_site/_ro/trn_rl_repo"):
    if os.path.isdir(_p) and _p not in sys.path:
        sys.path.insert(0, _p)

import numpy as np
import ml_dtypes

B, S, D, H = 4, 2048, 1024, 16
DK = 64
N_CORES = 8
HG = 2                  # head groups (cores per batch)
DH = D // HG            # 512: d_out per core
QC = 512                # q-chunk width per score matmul (one PSUM bank)
VB = 68                 # va block stride (64 vals + ones col + 3 pad)


def build_attention_nc(s=S, d=D, dh=DH, qc=QC):
    """Build the single-core Bass program (SPMD across 8 cores)."""
    import concourse.mybir as mybir
    import concourse.tile as tile
    from concourse import bacc

    f32 = mybir.dt.float32
    f16 = mybir.dt.float16
    bf16 = mybir.dt.bfloat16
    f8 = mybir.dt.float8e4
    EXPF = mybir.ActivationFunctionType.Exp

    n_h = dh // DK            # heads on this core (8)
    n_hp = n_h // 2           # head pairs (4)
    n_di = d // 128           # d_model 128-tiles (8)
    n_do = dh // 128          # d_out 128-tiles (4) == head pairs
    n_kt = s // 128           # key 128-tiles (16)
    n_qc = s // qc            # q chunks (4)
    n_st = s // 128           # seq 128-tiles (16)
    VA = n_h * VB             # va width per seq-tile

    nc = bacc.Bacc(None, target_bir_lowering=False)

    xqT = nc.dram_tensor("xqT", [d, s], f16, kind="ExternalInput")
    xkT = nc.dram_tensor("xkT", [d, s], f16, kind="ExternalInput")
    xvT = nc.dram_tensor("xvT", [d, s], f16, kind="ExternalInput")
    maskT = nc.dram_tensor("maskT", [s, s], f16, kind="ExternalInput")
    wq = nc.dram_tensor("wq", [d, dh], f16, kind="ExternalInput")
    wk = nc.dram_tensor("wk", [d, dh], f16, kind="ExternalInput")
    wv = nc.dram_tensor("wv", [d, dh], f16, kind="ExternalInput")
    wo = nc.dram_tensor("wo", [dh, d], f16, kind="ExternalInput")
    bqT = nc.dram_tensor("bqT", [128, n_do], f32, kind="ExternalInput")
    bkT = nc.dram_tensor("bkT", [128, n_do], f32, kind="ExternalInput")
    bv = nc.dram_tensor("bv", [1, dh], bf16, kind="ExternalInput")
    ones_d = nc.dram_tensor("ones_d", [1, 512], bf16, kind="ExternalInput")
    oT = nc.dram_tensor("oT", [d, s], f16, kind="ExternalOutput")

    # mask viewed as [p, kt, q] so one DMA grabs a [128, 4, qc] quarter
    maskT3 = maskT.rearrange("(kt p) q -> p kt q", p=128)

    scale = float(1.0 / np.sqrt(np.float32(DK)))

    # ---- engine-time estimates (ns) for the weave scheduler ----------
    C_SCORE_J = 300       # one j-unit: 2 concurrent K=64 matmuls N=512
    C_ATTNV_PAIR = 1030   # 4 matmuls N=512 M=65
    C_K_CHUNK = 2100      # 8 matmuls N=512 (one do, one seq quarter)
    C_Q_CHUNK = 2100      # 8 matmuls N=512 (one do, one qcb quarter)
    C_VP_CHUNK = 4200     # 16 matmuls N=512 (two seq tiles)
    C_OP_DM = 1030        # 4 matmuls N=512 (one out-proj d-tile)
    C_EXP_J = 1165        # ScalarE exp on [128, 1024] fp32
    MASK_LAT = 2200       # exp->mask pipeline margin before attn@V

    with tile.TileContext(nc) as tc:
        with (
            tc.tile_pool(name="xkv", bufs=32) as xkv_pool,
            tc.tile_pool(name="xq", bufs=16) as xq_pool,
            tc.tile_pool(name="w", bufs=16) as w_pool,
            tc.tile_pool(name="wv", bufs=8) as wv_pool,
            tc.tile_pool(name="wo", bufs=n_do) as wo_pool,
            tc.tile_pool(name="qk", bufs=2 * n_do) as qk_pool,
            tc.tile_pool(name="va", bufs=n_st) as va_pool,
            tc.tile_pool(name="ctxT", bufs=n_do) as ctxT_pool,
            tc.tile_pool(name="mask", bufs=5) as mask_pool,
            tc.tile_pool(name="e", bufs=8) as e_pool,
            tc.tile_pool(name="nrm", bufs=1) as nrm_pool,
            tc.tile_pool(name="nrmbc", bufs=1) as nrmbc_pool,
            tc.tile_pool(name="osb", bufs=2) as osb_pool,
            tc.tile_pool(name="const", bufs=1) as const_pool,
            tc.tile_pool(name="ps", bufs=2, space="PSUM") as ps_pool,
            tc.tile_pool(name="cps", bufs=3, space="PSUM") as c_pool,
            tc.tile_pool(name="ops", bufs=1, space="PSUM") as o_pool,
        ):
            # ---------------- constants ---------------------------------
            bqT_sb = const_pool.tile([128, n_do], f32, tag="biasq", name="bqT_sb")
            bkT_sb = const_pool.tile([128, n_do], f32, tag="biask", name="bkT_sb")
            nc.sync.dma_start(bqT_sb[:, :], bqT[:, :])
            nc.sync.dma_start(bkT_sb[:, :], bkT[:, :])

            # ---------------- input staging ------------------------------
            # xk / xv as [128, 512] seq-quarter tiles: K-projection (and the
            # first scores) can start after ~4MB of DMA instead of 7MB, and
            # frees recycle to xv at quarter granularity.
            wk_t, wq_t, wv_t = [], [], []
            xk_q = [[None] * n_di for _ in range(4)]
            xv_q = [[None] * n_di for _ in range(4)]
            xq_q = [[None] * n_di for _ in range(n_qc)]

            for di in range(n_di):
                wt = w_pool.tile([128, dh], f16, tag="w", name="wt")
                nc.sync.dma_start(wt[:, :], wk[di * 128:(di + 1) * 128, :])
                wk_t.append(wt)
                xt = xkv_pool.tile([128, qc], f16, tag="x", name="xt")
                nc.sync.dma_start(
                    xt[:, :], xkT[di * 128:(di + 1) * 128, 0:qc])
                xk_q[0][di] = xt
            for di in range(n_di):
                wt = w_pool.tile([128, dh], f16, tag="w", name="wt")
                nc.sync.dma_start(wt[:, :], wq[di * 128:(di + 1) * 128, :])
                wq_t.append(wt)
                xt = xq_pool.tile([128, qc], f16, tag="xq", name="xqt")
                nc.sync.dma_start(
                    xt[:, :], xqT[di * 128:(di + 1) * 128, 0:qc])
                xq_q[0][di] = xt

            mask_tiles = {}   # (qcb, quarter) -> tile [128, 4, qc]

            def emit_mask_dma(qcb, quarters=range(4)):
                for quarter in quarters:
                    mt = mask_pool.tile([128, 4, qc], f16, tag="m", name="mt")
                    nc.sync.dma_start(
                        mt[:, :, :],
                        maskT3[:, quarter * 4:(quarter + 1) * 4,
                               qcb * qc:(qcb + 1) * qc],
                    )
                    mask_tiles[(qcb, quarter)] = mt

            emit_mask_dma(0, range(0, 1))
            for qtr in range(1, 4):
                for di in range(n_di):
                    xt = xkv_pool.tile([128, qc], f16, tag="x", name="xt")
                    nc.sync.dma_start(
                        xt[:, :],
                        xkT[di * 128:(di + 1) * 128, qtr * qc:(qtr + 1) * qc])
                    xk_q[qtr][di] = xt
            for di in range(n_di):
                wt = wv_pool.tile([128, dh], f16, tag="wv", name="wvt")
                nc.sync.dma_start(wt[:, :], wv[di * 128:(di + 1) * 128, :])
                wv_t.append(wt)
            emit_mask_dma(0, range(1, 4))
            for di in range(n_di):
                xt = xq_pool.tile([128, qc], f16, tag="xq", name="xqt")
                nc.sync.dma_start(xt[:, :], xqT[di * 128:(di + 1) * 128,
                                                qc:2 * qc])
                xq_q[1][di] = xt
            wo_tiles = []
            for t in range(n_do):
                wt = wo_pool.tile([128, d], f16, tag="wo", name="wot")
                nc.sync.dma_start(wt[:, :], wo[t * 128:(t + 1) * 128, :])
                wo_tiles.append(wt)

            # xv and xq[2:3] DMAs reuse buffers freed by K-/Q-projection
            # matmuls.  A gated DMA parks at the head of the (in-order) DMA
            # queue and stalls everything behind it, which deadlocks if the
            # gate's producer is emitted later -- so these DMAs are emitted
            # from the scheduler, right after their producers.
            xv_staged = [False] * 4

            def stage_xv(qtr):
                for di in range(n_di):
                    xt = xkv_pool.tile([128, qc], f16, tag="x", name="xt")
                    nc.sync.dma_start(
                        xt[:, :], xvT[di * 128:(di + 1) * 128,
                                      qtr * qc:(qtr + 1) * qc])
                    xv_q[qtr][di] = xt
                xv_staged[qtr] = True

            def stage_xq(qcb):
                for di in range(n_di):
                    xt = xq_pool.tile([128, qc], f16, tag="xq", name="xqt")
                    nc.sync.dma_start(
                        xt[:, :],
                        xqT[di * 128:(di + 1) * 128, qcb * qc:(qcb + 1) * qc])
                    xq_q[qcb][di] = xt

            # ---------------- persistent SBUF tensors --------------------
            kT = [qk_pool.tile([128, s], bf16, tag="qk", name=f"kT{t}")
                  for t in range(n_do)]
            qT = [qk_pool.tile([128, s], bf16, tag="qk", name=f"qT{t}")
                  for t in range(n_do)]
            ctxT = [ctxT_pool.tile([128, s], f16, tag="ctxT", name=f"ctxT{t}")
                    for t in range(n_do)]
            va_tiles = [None] * n_st

            # ---------------- chunk emitters ------------------------------
            est = {"pe": 0.0, "act": 0.0}

            def emit_k_chunk(do, qtr):
                ps = ps_pool.tile([128, 1024], f32, tag="ps", name="ps")
                for di in range(n_di):
                    nc.tensor.matmul(
                        ps[:, 0:512],
                        wk_t[di][:, do * 128:(do + 1) * 128],
                        xk_q[qtr][di][:, :],
                        start=(di == 0), stop=(di == n_di - 1),
                    )
                nc.vector.tensor_scalar_add(
                    kT[do][:, qtr * qc:(qtr + 1) * qc],
                    ps[:, 0:512], bkT_sb[:, do:do + 1])
                est["pe"] += C_K_CHUNK

            def emit_q_chunk(do, qcb):
                ps = ps_pool.tile([128, 1024], f32, tag="ps", name="ps")
                for di in range(n_di):
                    nc.tensor.matmul(
                        ps[:, 0:512],
                        wq_t[di][:, do * 128:(do + 1) * 128],
                        xq_q[qcb][di][:, :],
                        start=(di == 0), stop=(di == n_di - 1),
                    )
                nc.vector.tensor_scalar_add(
                    qT[do][:, qcb * qc:(qcb + 1) * qc],
                    ps[:, 0:512], bqT_sb[:, do:do + 1])
                est["pe"] += C_Q_CHUNK

            def emit_vp_chunk(stp):
                vp = ps_pool.tile([128, 1024], f32, tag="ps", name="vp")
                for half in range(2):
                    st = stp * 2 + half
                    for di in range(n_di):
                        nc.tensor.matmul(
                            vp[:, half * 512:(half + 1) * 512],
                            xv_q[st // 4][di][:, (st % 4) * 128:
                                              (st % 4 + 1) * 128],
                            wv_t[di][:, :],
                            start=(di == 0), stop=(di == n_di - 1),
                        )
                for half in range(2):
                    st = stp * 2 + half
                    va = va_pool.tile([128, VA], f16, tag="va", name="va")
                    va3 = va.rearrange("p (h x) -> p h x", x=VB)
                    nc.vector.tensor_copy(
                        va3[:, :, 0:64],
                        vp[:, half * 512:(half + 1) * 512]
                        .rearrange("p (h x) -> p h x", x=64),
                    )
                    nc.gpsimd.memset(va3[:, :, 64:65], 1.0)
                    va_tiles[st] = va
                est["pe"] += C_VP_CHUNK

            pt_tiles = {}     # (qcb, hp, pair) -> masked-prob tile
            exp_ready = {}    # (qcb, hp, pair) -> est act ns when usable

            def emit_scores_pair(qcb, hp, pair):
                et = e_pool.tile([128, 2 * 1024], f16, tag="e", name="et")
                for j in range(2):
                    kt = pair * 2 + j
                    sp = ps_pool.tile([128, 1024], f32, tag="ps", name="sp")
                    for hh in range(2):
                        lo = hh * 64
                        nc.tensor.matmul(
                            sp[:, hh * qc:(hh + 1) * qc],
                            kT[hp][lo:lo + 64, kt * 128:(kt + 1) * 128],
                            qT[hp][lo:lo + 64, qcb * qc:(qcb + 1) * qc],
                            start=True, stop=True,
                        )
                    est["pe"] += C_SCORE_J
                    nc.scalar.activation(
                        et[:, j * 1024:(j + 1) * 1024], sp[:, :], EXPF,
                        scale=scale)
                    est["act"] = max(est["act"], est["pe"]) + C_EXP_J
                mt = mask_tiles[(qcb, pair // 2)]
                m4 = (mt[:, (pair % 2) * 2:(pair % 2) * 2 + 2, :]
                      .unsqueeze(2).broadcast_to([128, 2, 2, qc]))
                nc.vector.tensor_mul(
                    et[:, :].rearrange("p (k h q) -> p k h q", k=2, q=qc),
                    et[:, :].rearrange("p (k h q) -> p k h q", k=2, q=qc),
                    m4)
                pt_tiles[(qcb, hp, pair)] = et
                exp_ready[(qcb, hp, pair)] = est["act"] + MASK_LAT

            cps_cur = {}      # live ctx psum tiles for the active unit

            def emit_attnv_pair(qcb, hp, pair):
                if pair == 0:
                    cps_cur["t"] = [
                        c_pool.tile([128, qc], f32, tag="c", name="cp")
                        for _ in range(2)]
                cps = cps_cur["t"]
                pt = pt_tiles.pop((qcb, hp, pair))
                for j in range(2):
                    kt = pair * 2 + j
                    for hh in range(2):
                        h = hp * 2 + hh
                        nc.tensor.matmul(
                            cps[hh][0:65, :],
                            va_tiles[kt][:, h * VB:h * VB + 65],
                            pt[:, j * 1024 + hh * qc:
                               j * 1024 + (hh + 1) * qc],
                            start=(kt == 0), stop=(kt == n_kt - 1),
                            skip_group_check=True,
                        )
                est["pe"] += C_ATTNV_PAIR

            def emit_norm(qcb, hp):
                cps = cps_cur.pop("t")
                for hh in range(2):
                    lt = nrm_pool.tile([1, qc], f32, tag="l", name="lt")
                    nc.vector.tensor_copy(lt[0:1, :], cps[hh][64:65, :])
                    rt = nrm_pool.tile([1, qc], f32, tag="r", name="rt")
                    nc.vector.reciprocal_approx_fast(rt[0:1, :], lt[0:1, :])
                    bc = nrmbc_pool.tile([64, qc], f32, tag="bc", name="bc")
                    nc.gpsimd.partition_broadcast(bc[:, :], rt[0:1, :], 64)
                    nc.vector.tensor_mul(
                        ctxT[hp][hh * 64:hh * 64 + 64, qcb * qc:(qcb + 1) * qc],
                        cps[hh][0:64, :], bc[:, :])

            def emit_outproj_dm(qcb, dm, pool=None, tag=None):
                op = (pool or o_pool).tile([128, 512], f32,
                                           tag=tag or "o", name="ops")
                for t in range(n_do):
                    nc.tensor.matmul(
                        op[:, :],
                        wo_tiles[t][:, dm * 128:(dm + 1) * 128],
                        ctxT[t][:, qcb * qc:(qcb + 1) * qc],
                        start=(t == 0), stop=(t == n_do - 1),
                    )
                osb = osb_pool.tile([128, 512], f16, tag="osb", name="osb")
                nc.vector.tensor_copy(osb[:, :], op[:, :])
                nc.sync.dma_start(
                    oT[dm * 128:(dm + 1) * 128, qcb * qc:(qcb + 1) * qc],
                    osb[:, :],
                )
                est["pe"] += C_OP_DM

            # ---------------- weave scheduler -----------------------------
            units = [(qcb, hp) for qcb in range(n_qc) for hp in range(n_hp)]

            # F1: projection chain, deadline-sorted in (unit, pair) space.
            # K(hp, qtr) is first needed by pair 2*qtr of unit (0, hp).
            F1 = [(("K", 0, qtr), (0, 2 * qtr)) for qtr in range(1, 4)]
            for hp in range(1, n_hp):
                F1.append((("K", hp, 0), (hp, 0)))
                F1.append((("Q", hp, 0), (hp, 0)))
                for qtr in range(1, 4):
                    F1.append((("K", hp, qtr), (hp, 2 * qtr)))
            # Q(do, qcb) deadline sits at pair 6 of the unit BEFORE its
            # first consumer: forcing it there overlaps the 2us projection
            # block with ScalarE's remaining backlog instead of opening a
            # gap right at the unit boundary.
            for qcb in range(1, n_qc):
                for do in range(n_do):
                    F1.append((("Q", do, qcb), (qcb * n_hp + do - 1, 6)))

            vp_next = [0]          # next vp chunk (0..7)
            # attn@V stream: per unit: 8 pairs then norm
            av_steps = []
            for u, (qcb, hp) in enumerate(units):
                for pair in range(n_kt // 2):
                    av_steps.append(("pair", qcb, hp, pair, u))
                av_steps.append(("norm", qcb, hp, u))
            av_i = [0]
            norms_done = [0] * n_qc
            F3 = []                # ready out-proj (qcb, dm) chunks

            def emit_f1_item(item):
                kind, a, b = item
                if kind == "K":
                    emit_k_chunk(a, b)
                    if a == n_do - 1:          # last reader of xk quarter b
                        stage_xv(b)
                else:
                    emit_q_chunk(a, b)
                    if a == n_do - 1 and b < 2:  # last reader of xq[b]
                        stage_xq(b + 2)

            def ensure_xv(qtr):
                # vp chunk needs xv staged; pop F1 until K(n_do-1, qtr)
                # has been emitted (it always sits in F1)
                while not xv_staged[qtr]:
                    emit_f1_item(F1.pop(0)[0])

            def attnv_step_ready(step, cur_unit_idx):
                if step[0] == "norm":
                    return True
                _, qcb, hp, pair, u = step
                r = exp_ready.get((qcb, hp, pair))
                if r is None or r > est["pe"]:
                    return False
                return vp_next[0] > pair       # va[2p], va[2p+1] staged

            def emit_attnv_step(step):
                if step[0] == "norm":
                    _, qcb, hp, u = step
                    emit_norm(qcb, hp)
                    norms_done[qcb] += 1
                    if norms_done[qcb] == n_hp and qcb < n_qc - 1:
                        for dm in range(n_di):
                            F3.append((qcb, dm))
                else:
                    _, qcb, hp, pair, u = step
                    emit_attnv_pair(qcb, hp, pair)
                av_i[0] += 1

            def pick_and_emit(cur_unit_idx):
                """Emit one filler chunk; return False if nothing to do."""
                if av_i[0] < len(av_steps):
                    step = av_steps[av_i[0]]
                    if attnv_step_ready(step, cur_unit_idx):
                        emit_attnv_step(step)
                        return True
                    if (step[0] == "pair" and vp_next[0] <= step[3]
                            and vp_next[0] < n_st // 2
                            and exp_ready.get(tuple(step[1:4]), 1e18)
                            <= est["pe"] + C_VP_CHUNK):
                        ensure_xv(vp_next[0] // 2)
                        emit_vp_chunk(vp_next[0])
                        vp_next[0] += 1
                        return True
                if F1 and F1[0][1] <= (cur_unit_idx + 2, 99):
                    emit_f1_item(F1.pop(0)[0])
                    return True
                if F3:
                    qcb, dm = F3.pop(0)
                    emit_outproj_dm(qcb, dm)
                    return True
                if F1:
                    emit_f1_item(F1.pop(0)[0])
                    return True
                if vp_next[0] < n_st // 2 and xv_staged[vp_next[0] // 4]:
                    emit_vp_chunk(vp_next[0])
                    vp_next[0] += 1
                    return True
                return False

            def force_drain(limit):
                # keep the spine at most `limit` un-consumed prob tiles
                # ahead of the attn@V stream (PE-queue deadlock guard:
                # an exhausted e-pool stalls exp behind attn@V matmuls
                # that would otherwise be emitted after the next scores)
                while len(pt_tiles) >= limit and av_i[0] < len(av_steps):
                    step = av_steps[av_i[0]]
                    if step[0] == "pair" and vp_next[0] <= step[3]:
                        ensure_xv(vp_next[0] // 2)
                        emit_vp_chunk(vp_next[0])
                        vp_next[0] += 1
                        continue
                    emit_attnv_step(step)

            # head: minimal projections to start the exp stream (kT/qT
            # for unit (0,0) pairs 0-1 only -- ~4MB of DMA)
            emit_k_chunk(0, 0)
            emit_q_chunk(0, 0)

            for ui, (qcb, hp) in enumerate(units):
                if hp == 0 and qcb > 0:
                    emit_mask_dma(qcb, range(1, 4))
                if hp == 3 and qcb + 1 < n_qc:
                    emit_mask_dma(qcb + 1, range(0, 1))
                for pair in range(n_kt // 2):
                    # force any overdue projection prerequisites
                    while F1 and F1[0][1] <= (ui, pair):
                        emit_f1_item(F1.pop(0)[0])
                    force_drain(6)
                    emit_scores_pair(qcb, hp, pair)
                    while est["pe"] < est["act"]:
                        if not pick_and_emit(ui):
                            break

            # drain remaining attn@V / norms / fillers
            while av_i[0] < len(av_steps):
                step = av_steps[av_i[0]]
                if (step[0] == "pair" and vp_next[0] <= step[3]):
                    ensure_xv(vp_next[0] // 2)
                    emit_vp_chunk(vp_next[0])
                    vp_next[0] += 1
                    continue
                emit_attnv_step(step)
            while F3:
                qcb, dm = F3.pop(0)
                emit_outproj_dm(qcb, dm)

            # final qcb out-proj: rotate through the idle cps ring for
            # pipelined evacuation (attention is done).
            qcb = n_qc - 1

            def close_dm(dm, op):
                nc.tensor.matmul(
                    op[:, :],
                    wo_tiles[n_do - 1][:, dm * 128:(dm + 1) * 128],
                    ctxT[n_do - 1][:, qcb * qc:(qcb + 1) * qc],
                    start=False, stop=True,
                )
                osb = osb_pool.tile([128, 512], f16, tag="osb", name="osb")
                nc.vector.tensor_copy(osb[:, :], op[:, :])
                nc.sync.dma_start(
                    oT[dm * 128:(dm + 1) * 128, qcb * qc:(qcb + 1) * qc],
                    osb[:, :],
                )

            pend = []
            for dm in range(n_di):
                if len(pend) == 4:
                    close_dm(*pend.pop(0))
                op = (o_pool if dm == 0 else c_pool).tile(
                    [128, 512], f32, tag="o" if dm == 0 else "c", name="ops")
                for t in range(n_do - 1):
                    nc.tensor.matmul(
                        op[:, :],
                        wo_tiles[t][:, dm * 128:(dm + 1) * 128],
                        ctxT[t][:, qcb * qc:(qcb + 1) * qc],
                        start=(t == 0), stop=False,
                    )
                pend.append((dm, op))
            for dm, op in pend:
                close_dm(dm, op)

    nc.compile()
    return nc


def make_in_maps(Q, K, V, mask, Wq, bq, Wk, bk, Wv, bv, Wo):
    Q = np.asarray(Q, np.float32)
    K = np.asarray(K, np.float32)
    V = np.asarray(V, np.float32)
    mask = np.asarray(mask)
    n_do = DH // 128
    in_maps = []
    for c in range(N_CORES):
        b, hg = c // HG, c % HG
        cs = slice(hg * DH, (hg + 1) * DH)
        in_maps.append({
            "xqT": np.ascontiguousarray(Q[b].T).astype(np.float16),
            "xkT": np.ascontiguousarray(K[b].T).astype(np.float16),
            "xvT": np.ascontiguousarray(V[b].T).astype(np.float16),
            "maskT": np.ascontiguousarray(mask[b, 0].T).astype(np.float16),
            "wq": np.asarray(Wq, np.float32)[:, cs].astype(np.float16),
            "wk": np.asarray(Wk, np.float32)[:, cs].astype(np.float16),
            "wv": np.asarray(Wv, np.float32)[:, cs].astype(np.float16),
            "wo": np.asarray(Wo, np.float32)[cs, :].astype(np.float16),
            "bqT": np.ascontiguousarray(
                np.asarray(bq, np.float32)[cs].reshape(n_do, 128).T),
            "bkT": np.ascontiguousarray(
                np.asarray(bk, np.float32)[cs].reshape(n_do, 128).T),
            "bv": np.asarray(bv, np.float32)[cs].reshape(1, DH)
                .astype(ml_dtypes.bfloat16),
            "ones_d": np.ones((1, 512), ml_dtypes.bfloat16),
        })
    return in_maps


def combine_outputs(results, bo):
    out = np.empty((B, S, D), np.float32)
    for b in range(B):
        out[b] = (results[HG * b]["oT"].astype(np.float32)
                  + results[HG * b + 1]["oT"].astype(np.float32)).T
    out += np.asarray(bo, np.float32)
    return out


def kernel(Q, K, V, mask, Wq, bq, Wk, bk, Wv, bv, Wo, bo):
    from concourse.bass_utils import run_bass_kernel_spmd

    in_maps = make_in_maps(Q, K, V, mask, Wq, bq, Wk, bk, Wv, bv, Wo)
    nc = build_attention_nc()
    res = run_bass_kernel_spmd(nc, in_maps, core_ids=list(range(N_CORES)))
    return combine_outputs(res.results, bo)


# revision 28
# speedup vs baseline: 1.1910x; 1.1910x over previous
"""Multi-head attention Trainium2 kernel (v3).

Full inputs -> shard over 8 NeuronCores (batch x head-group) -> full output.

Per core c: batch b = c // 2, head-group hg = c % 2 (8 of 16 heads).
Column-shard Wq/Wk/Wv, row-shard Wo; each core computes a partial output
projection for its batch; host sums the two partials per batch and adds bo.

v3: estimate-driven dense weave.  The softmax exp stream on ScalarE
(256 x ~1.15us) is the steady-state floor, so the schedule starts it as
early as possible (K0+Q0-quarter projections only) and then interleaves
all remaining PE work (K/Q projections, V-proj, attn@V, out-proj) into
the gaps between score matmul pairs, paced by a ns-level estimate of
each engine's backlog.  This keeps ScalarE ~always busy and the PE
stream dense (no HAM cold-throttle oscillation).

Layout (per core), same math as v2:
  - scores^T [k, q] with two heads of a pair packed on PE row halves
    (concurrent K=64 matmuls); exp on ScalarE; mask multiply on DVE.
  - V staged as [seq, 8 heads x 68] fp16 blocks: cols 0-63 = V, col 64 =
    ones (softmax denominators via the attn@V matmul), 65-67 pad.
  - normalization: reciprocal_approx_fast + gpsimd partition_broadcast
    + one DVE multiply into ctxT.
  - PSUM: shared proj/scores pool 2x[128,1024] (4 banks) + ctx
    3x[128,512] (3 banks) + out-proj 1x[128,512] (1 bank) = 8 banks.
"""

import os
import sys

for _p in ("/opt/trn_rl_repo", "/root/.axon_site/_ro/trn_rl_repo"):
    if os.path.isdir(_p) and _p not in sys.path:
        sys.path.insert(0, _p)

import numpy as np
import ml_dtypes

B, S, D, H = 4, 2048, 1024, 16
DK = 64
N_CORES = 8
HG = 2                  # head groups (cores per batch)
DH = D // HG            # 512: d_out per core
QC = 512                # q-chunk width per score matmul (one PSUM bank)
VB = 68                 # va block stride (64 vals + ones col + 3 pad)


def build_attention_nc(s=S, d=D, dh=DH, qc=QC):
    """Build the single-core Bass program (SPMD across 8 cores)."""
    import concourse.mybir as mybir
    import concourse.tile as tile
    from concourse import bacc

    f32 = mybir.dt.float32
    f16 = mybir.dt.float16
    bf16 = mybir.dt.bfloat16
    f8 = mybir.dt.float8e4
    EXPF = mybir.ActivationFunctionType.Exp

    n_h = dh // DK            # heads on this core (8)
    n_hp = n_h // 2           # head pairs (4)
    n_di = d // 128           # d_model 128-tiles (8)
    n_do = dh // 128          # d_out 128-tiles (4) == head pairs
    n_kt = s // 128           # key 128-tiles (16)
    n_qc = s // qc            # q chunks (4)
    n_st = s // 128           # seq 128-tiles (16)
    VA = n_h * VB             # va width per seq-tile

    nc = bacc.Bacc(None, target_bir_lowering=False)

    xqT = nc.dram_tensor("xqT", [d, s], f16, kind="ExternalInput")
    xkT = nc.dram_tensor("xkT", [d, s], f16, kind="ExternalInput")
    xvT = nc.dram_tensor("xvT", [d, s], f16, kind="ExternalInput")
    maskT = nc.dram_tensor("maskT", [s, s], f16, kind="ExternalInput")
    wq = nc.dram_tensor("wq", [d, dh], f16, kind="ExternalInput")
    wk = nc.dram_tensor("wk", [d, dh], f16, kind="ExternalInput")
    wv = nc.dram_tensor("wv", [d, dh], f16, kind="ExternalInput")
    wo = nc.dram_tensor("wo", [dh, d], f16, kind="ExternalInput")
    bqT = nc.dram_tensor("bqT", [128, n_do], f32, kind="ExternalInput")
    bkT = nc.dram_tensor("bkT", [128, n_do], f32, kind="ExternalInput")
    bv = nc.dram_tensor("bv", [1, dh], bf16, kind="ExternalInput")
    ones_d = nc.dram_tensor("ones_d", [1, 512], bf16, kind="ExternalInput")
    oT = nc.dram_tensor("oT", [d, s], f16, kind="ExternalOutput")

    # mask viewed as [p, kt, q] so one DMA grabs a [128, 4, qc] quarter
    maskT3 = maskT.rearrange("(kt p) q -> p kt q", p=128)

    scale = float(1.0 / np.sqrt(np.float32(DK)))

    # ---- engine-time estimates (ns) for the weave scheduler ----------
    C_SCORE_J = 300       # one j-unit: 2 concurrent K=64 matmuls N=512
    C_ATTNV_PAIR = 1030   # 4 matmuls N=512 M=65
    C_K_CHUNK = 2100      # 8 matmuls N=512 (one do, one seq quarter)
    C_Q_CHUNK = 2100      # 8 matmuls N=512 (one do, one qcb quarter)
    C_VP_CHUNK = 4200     # 16 matmuls N=512 (two seq tiles)
    C_OP_DM = 1030        # 4 matmuls N=512 (one out-proj d-tile)
    C_EXP_J = 1165        # ScalarE exp on [128, 1024] fp32
    MASK_LAT = 2200       # exp->mask pipeline margin before attn@V

    with tile.TileContext(nc) as tc:
        with (
            tc.tile_pool(name="xkv", bufs=32) as xkv_pool,
            tc.tile_pool(name="xq", bufs=16) as xq_pool,
            tc.tile_pool(name="w", bufs=16) as w_pool,
            tc.tile_pool(name="wv", bufs=8) as wv_pool,
            tc.tile_pool(name="wo", bufs=n_do) as wo_pool,
            tc.tile_pool(name="qk", bufs=2 * n_do) as qk_pool,
            tc.tile_pool(name="va", bufs=n_st) as va_pool,
            tc.tile_pool(name="ctxT", bufs=n_do) as ctxT_pool,
            tc.tile_pool(name="mask", bufs=5) as mask_pool,
            tc.tile_pool(name="e", bufs=8) as e_pool,
            tc.tile_pool(name="nrm", bufs=1) as nrm_pool,
            tc.tile_pool(name="nrmbc", bufs=1) as nrmbc_pool,
            tc.tile_pool(name="osb", bufs=2) as osb_pool,
            tc.tile_pool(name="const", bufs=1) as const_pool,
            tc.tile_pool(name="ps", bufs=2, space="PSUM") as ps_pool,
            tc.tile_pool(name="cps", bufs=3, space="PSUM") as c_pool,
            tc.tile_pool(name="ops", bufs=1, space="PSUM") as o_pool,
        ):
            # ---------------- constants ---------------------------------
            bqT_sb = const_pool.tile([128, n_do], f32, tag="biasq", name="bqT_sb")
            bkT_sb = const_pool.tile([128, n_do], f32, tag="biask", name="bkT_sb")
            nc.sync.dma_start(bqT_sb[:, :], bqT[:, :])
            nc.sync.dma_start(bkT_sb[:, :], bkT[:, :])

            # ---------------- input staging ------------------------------
            # xk / xv as [128, 512] seq-quarter tiles: K-projection (and the
            # first scores) can start after ~4MB of DMA instead of 7MB, and
            # frees recycle to xv at quarter granularity.
            wk_t, wq_t, wv_t = [], [], []
            xk_q = [[None] * n_di for _ in range(4)]
            xv_q = [[None] * n_di for _ in range(4)]
            xq_q = [[None] * n_di for _ in range(n_qc)]

            for di in range(n_di):
                wt = w_pool.tile([128, dh], f16, tag="w", name="wt")
                nc.sync.dma_start(wt[:, :], wk[di * 128:(di + 1) * 128, :])
                wk_t.append(wt)
                xt = xkv_pool.tile([128, qc], f16, tag="x", name="xt")
                nc.sync.dma_start(
                    xt[:, :], xkT[di * 128:(di + 1) * 128, 0:qc])
                xk_q[0][di] = xt
            for di in range(n_di):
                wt = w_pool.tile([128, dh], f16, tag="w", name="wt")
                nc.sync.dma_start(wt[:, :], wq[di * 128:(di + 1) * 128, :])
                wq_t.append(wt)
                xt = xq_pool.tile([128, qc], f16, tag="xq", name="xqt")
                nc.sync.dma_start(
                    xt[:, :], xqT[di * 128:(di + 1) * 128, 0:qc])
                xq_q[0][di] = xt

            mask_tiles = {}   # (qcb, quarter) -> tile [128, 4, qc]

            def emit_mask_dma(qcb, quarters=range(4)):
                for quarter in quarters:
                    mt = mask_pool.tile([128, 4, qc], f16, tag="m", name="mt")
                    nc.sync.dma_start(
                        mt[:, :, :],
                        maskT3[:, quarter * 4:(quarter + 1) * 4,
                               qcb * qc:(qcb + 1) * qc],
                    )
                    mask_tiles[(qcb, quarter)] = mt

            emit_mask_dma(0, range(0, 1))
            for qtr in range(1, 4):
                for di in range(n_di):
                    xt = xkv_pool.tile([128, qc], f16, tag="x", name="xt")
                    nc.sync.dma_start(
                        xt[:, :],
                        xkT[di * 128:(di + 1) * 128, qtr * qc:(qtr + 1) * qc])
                    xk_q[qtr][di] = xt
            for di in range(n_di):
                wt = wv_pool.tile([128, dh], f16, tag="wv", name="wvt")
                nc.sync.dma_start(wt[:, :], wv[di * 128:(di + 1) * 128, :])
                wv_t.append(wt)
            emit_mask_dma(0, range(1, 4))
            for di in range(n_di):
                xt = xq_pool.tile([128, qc], f16, tag="xq", name="xqt")
                nc.sync.dma_start(xt[:, :], xqT[di * 128:(di + 1) * 128,
                                                qc:2 * qc])
                xq_q[1][di] = xt
            wo_tiles = []
            for t in range(n_do):
                wt = wo_pool.tile([128, d], f16, tag="wo", name="wot")
                nc.sync.dma_start(wt[:, :], wo[t * 128:(t + 1) * 128, :])
                wo_tiles.append(wt)

            # xv and xq[2:3] DMAs reuse buffers freed by K-/Q-projection
            # matmuls.  A gated DMA parks at the head of the (in-order) DMA
            # queue and stalls everything behind it, which deadlocks if the
            # gate's producer is emitted later -- so these DMAs are emitted
            # from the scheduler, right after their producers.
            xv_staged = [False] * 4

            def stage_xv(qtr):
                for di in range(n_di):
                    xt = xkv_pool.tile([128, qc], f16, tag="x", name="xt")
                    nc.sync.dma_start(
                        xt[:, :], xvT[di * 128:(di + 1) * 128,
                                      qtr * qc:(qtr + 1) * qc])
                    xv_q[qtr][di] = xt
                xv_staged[qtr] = True

            def stage_xq(qcb):
                for di in range(n_di):
                    xt = xq_pool.tile([128, qc], f16, tag="xq", name="xqt")
                    nc.sync.dma_start(
                        xt[:, :],
                        xqT[di * 128:(di + 1) * 128, qcb * qc:(qcb + 1) * qc])
                    xq_q[qcb][di] = xt

            # ---------------- persistent SBUF tensors --------------------
            kT = [qk_pool.tile([128, s], bf16, tag="qk", name=f"kT{t}")
                  for t in range(n_do)]
            qT = [qk_pool.tile([128, s], bf16, tag="qk", name=f"qT{t}")
                  for t in range(n_do)]
            ctxT = [ctxT_pool.tile([128, s], f16, tag="ctxT", name=f"ctxT{t}")
                    for t in range(n_do)]
            va_tiles = [None] * n_st

            # ---------------- chunk emitters ------------------------------
            est = {"pe": 0.0, "act": 0.0}

            def emit_k_chunk(do, qtr):
                ps = ps_pool.tile([128, 1024], f32, tag="ps", name="ps")
                for di in range(n_di):
                    nc.tensor.matmul(
                        ps[:, 0:512],
                        wk_t[di][:, do * 128:(do + 1) * 128],
                        xk_q[qtr][di][:, :],
                        start=(di == 0), stop=(di == n_di - 1),
                    )
                nc.vector.tensor_scalar_add(
                    kT[do][:, qtr * qc:(qtr + 1) * qc],
                    ps[:, 0:512], bkT_sb[:, do:do + 1])
                est["pe"] += C_K_CHUNK

            def emit_q_chunk(do, qcb):
                ps = ps_pool.tile([128, 1024], f32, tag="ps", name="ps")
                for di in range(n_di):
                    nc.tensor.matmul(
                        ps[:, 0:512],
                        wq_t[di][:, do * 128:(do + 1) * 128],
                        xq_q[qcb][di][:, :],
                        start=(di == 0), stop=(di == n_di - 1),
                    )
                nc.vector.tensor_scalar_add(
                    qT[do][:, qcb * qc:(qcb + 1) * qc],
                    ps[:, 0:512], bqT_sb[:, do:do + 1])
                est["pe"] += C_Q_CHUNK

            def emit_vp_chunk(stp):
                vp = ps_pool.tile([128, 1024], f32, tag="ps", name="vp")
                for half in range(2):
                    st = stp * 2 + half
                    for di in range(n_di):
                        nc.tensor.matmul(
                            vp[:, half * 512:(half + 1) * 512],
                            xv_q[st // 4][di][:, (st % 4) * 128:
                                              (st % 4 + 1) * 128],
                            wv_t[di][:, :],
                            start=(di == 0), stop=(di == n_di - 1),
                        )
                for half in range(2):
                    st = stp * 2 + half
                    va = va_pool.tile([128, VA], f16, tag="va", name="va")
                    va3 = va.rearrange("p (h x) -> p h x", x=VB)
                    nc.vector.tensor_copy(
                        va3[:, :, 0:64],
                        vp[:, half * 512:(half + 1) * 512]
                        .rearrange("p (h x) -> p h x", x=64),
                    )
                    nc.gpsimd.memset(va3[:, :, 64:65], 1.0)
                    va_tiles[st] = va
                est["pe"] += C_VP_CHUNK

            pt_tiles = {}     # (qcb, hp, pair) -> masked-prob tile
            exp_ready = {}    # (qcb, hp, pair) -> est act ns when usable

            def emit_scores_pair(qcb, hp, pair):
                et = e_pool.tile([128, 2 * 1024], f16, tag="e", name="et")
                for j in range(2):
                    kt = pair * 2 + j
                    sp = ps_pool.tile([128, 1024], f32, tag="ps", name="sp")
                    for hh in range(2):
                        lo = hh * 64
                        nc.tensor.matmul(
                            sp[:, hh * qc:(hh + 1) * qc],
                            kT[hp][lo:lo + 64, kt * 128:(kt + 1) * 128],
                            qT[hp][lo:lo + 64, qcb * qc:(qcb + 1) * qc],
                            start=True, stop=True,
                        )
                    est["pe"] += C_SCORE_J
                    nc.scalar.activation(
                        et[:, j * 1024:(j + 1) * 1024], sp[:, :], EXPF,
                        scale=scale)
                    est["act"] = max(est["act"], est["pe"]) + C_EXP_J
                mt = mask_tiles[(qcb, pair // 2)]
                m4 = (mt[:, (pair % 2) * 2:(pair % 2) * 2 + 2, :]
                      .unsqueeze(2).broadcast_to([128, 2, 2, qc]))
                nc.vector.tensor_mul(
                    et[:, :].rearrange("p (k h q) -> p k h q", k=2, q=qc),
                    et[:, :].rearrange("p (k h q) -> p k h q", k=2, q=qc),
                    m4)
                pt_tiles[(qcb, hp, pair)] = et
                exp_ready[(qcb, hp, pair)] = est["act"] + MASK_LAT

            cps_cur = {}      # live ctx psum tiles for the active unit

            def emit_attnv_pair(qcb, hp, pair):
                if pair == 0:
                    cps_cur["t"] = [
                        c_pool.tile([128, qc], f32, tag="c", name="cp")
                        for _ in range(2)]
                cps = cps_cur["t"]
                pt = pt_tiles.pop((qcb, hp, pair))
                for j in range(2):
                    kt = pair * 2 + j
                    for hh in range(2):
                        h = hp * 2 + hh
                        nc.tensor.matmul(
                            cps[hh][0:65, :],
                            va_tiles[kt][:, h * VB:h * VB + 65],
                            pt[:, j * 1024 + hh * qc:
                               j * 1024 + (hh + 1) * qc],
                            start=(kt == 0), stop=(kt == n_kt - 1),
                            skip_group_check=True,
                        )
                est["pe"] += C_ATTNV_PAIR

            def emit_norm(qcb, hp):
                cps = cps_cur.pop("t")
                for hh in range(2):
                    lt = nrm_pool.tile([1, qc], f32, tag="l", name="lt")
                    nc.vector.tensor_copy(lt[0:1, :], cps[hh][64:65, :])
                    rt = nrm_pool.tile([1, qc], f32, tag="r", name="rt")
                    nc.vector.reciprocal_approx_fast(rt[0:1, :], lt[0:1, :])
                    bc = nrmbc_pool.tile([64, qc], f32, tag="bc", name="bc")
                    nc.gpsimd.partition_broadcast(bc[:, :], rt[0:1, :], 64)
                    nc.vector.tensor_mul(
                        ctxT[hp][hh * 64:hh * 64 + 64, qcb * qc:(qcb + 1) * qc],
                        cps[hh][0:64, :], bc[:, :])

            def emit_outproj_dm(qcb, dm, pool=None, tag=None):
                op = (pool or o_pool).tile([128, 512], f32,
                                           tag=tag or "o", name="ops")
                for t in range(n_do):
                    nc.tensor.matmul(
                        op[:, :],
                        wo_tiles[t][:, dm * 128:(dm + 1) * 128],
                        ctxT[t][:, qcb * qc:(qcb + 1) * qc],
                        start=(t == 0), stop=(t == n_do - 1),
                    )
                osb = osb_pool.tile([128, 512], f16, tag="osb", name="osb")
                nc.vector.tensor_copy(osb[:, :], op[:, :])
                nc.sync.dma_start(
                    oT[dm * 128:(dm + 1) * 128, qcb * qc:(qcb + 1) * qc],
                    osb[:, :],
                )
                est["pe"] += C_OP_DM

            # ---------------- weave scheduler -----------------------------
            units = [(qcb, hp) for qcb in range(n_qc) for hp in range(n_hp)]

            # F1: projection chain, deadline-sorted in (unit, pair) space.
            # K(hp, qtr) is first needed by pair 2*qtr of unit (0, hp).
            F1 = [(("K", 0, qtr), (0, 2 * qtr)) for qtr in range(1, 4)]
            for hp in range(1, n_hp):
                F1.append((("K", hp, 0), (hp, 0)))
                F1.append((("Q", hp, 0), (hp, 0)))
                for qtr in range(1, 4):
                    F1.append((("K", hp, qtr), (hp, 2 * qtr)))
            for qcb in range(1, n_qc):
                for do in range(n_do):
                    F1.append((("Q", do, qcb), (qcb * n_hp + do, 0)))

            vp_next = [0]          # next vp chunk (0..7)
            # attn@V stream: per unit: 8 pairs then norm
            av_steps = []
            for u, (qcb, hp) in enumerate(units):
                for pair in range(n_kt // 2):
                    av_steps.append(("pair", qcb, hp, pair, u))
                av_steps.append(("norm", qcb, hp, u))
            av_i = [0]
            norms_done = [0] * n_qc
            F3 = []                # ready out-proj (qcb, dm) chunks

            def emit_f1_item(item):
                kind, a, b = item
                if kind == "K":
                    emit_k_chunk(a, b)
                    if a == n_do - 1:          # last reader of xk quarter b
                        stage_xv(b)
                else:
                    emit_q_chunk(a, b)
                    if a == n_do - 1 and b < 2:  # last reader of xq[b]
                        stage_xq(b + 2)

            def ensure_xv(qtr):
                # vp chunk needs xv staged; pop F1 until K(n_do-1, qtr)
                # has been emitted (it always sits in F1)
                while not xv_staged[qtr]:
                    emit_f1_item(F1.pop(0)[0])

            def attnv_step_ready(step, cur_unit_idx):
                if step[0] == "norm":
                    return True
                _, qcb, hp, pair, u = step
                r = exp_ready.get((qcb, hp, pair))
                if r is None or r > est["pe"]:
                    return False
                return vp_next[0] > pair       # va[2p], va[2p+1] staged

            def emit_attnv_step(step):
                if step[0] == "norm":
                    _, qcb, hp, u = step
                    emit_norm(qcb, hp)
                    norms_done[qcb] += 1
                    if norms_done[qcb] == n_hp and qcb < n_qc - 1:
                        for dm in range(n_di):
                            F3.append((qcb, dm))
                else:
                    _, qcb, hp, pair, u = step
                    emit_attnv_pair(qcb, hp, pair)
                av_i[0] += 1

            def pick_and_emit(cur_unit_idx):
                """Emit one filler chunk; return False if nothing to do."""
                if av_i[0] < len(av_steps):
                    step = av_steps[av_i[0]]
                    if attnv_step_ready(step, cur_unit_idx):
                        emit_attnv_step(step)
                        return True
                    if (step[0] == "pair" and vp_next[0] <= step[3]
                            and vp_next[0] < n_st // 2
                            and exp_ready.get(tuple(step[1:4]), 1e18)
                            <= est["pe"] + C_VP_CHUNK):
                        ensure_xv(vp_next[0] // 2)
                        emit_vp_chunk(vp_next[0])
                        vp_next[0] += 1
                        return True
                if F1 and F1[0][1] <= (cur_unit_idx + 2, 99):
                    emit_f1_item(F1.pop(0)[0])
                    return True
                if F3:
                    qcb, dm = F3.pop(0)
                    emit_outproj_dm(qcb, dm)
                    return True
                if F1:
                    emit_f1_item(F1.pop(0)[0])
                    return True
                if vp_next[0] < n_st // 2 and xv_staged[vp_next[0] // 4]:
                    emit_vp_chunk(vp_next[0])
                    vp_next[0] += 1
                    return True
                return False

            def force_drain(limit):
                # keep the spine at most `limit` un-consumed prob tiles
                # ahead of the attn@V stream (PE-queue deadlock guard:
                # an exhausted e-pool stalls exp behind attn@V matmuls
                # that would otherwise be emitted after the next scores)
                while len(pt_tiles) >= limit and av_i[0] < len(av_steps):
                    step = av_steps[av_i[0]]
                    if step[0] == "pair" and vp_next[0] <= step[3]:
                        ensure_xv(vp_next[0] // 2)
                        emit_vp_chunk(vp_next[0])
                        vp_next[0] += 1
                        continue
                    emit_attnv_step(step)

            # head: minimal projections to start the exp stream (kT/qT
            # for unit (0,0) pairs 0-1 only -- ~4MB of DMA)
            emit_k_chunk(0, 0)
            emit_q_chunk(0, 0)

            for ui, (qcb, hp) in enumerate(units):
                if hp == 0 and qcb > 0:
                    emit_mask_dma(qcb, range(1, 4))
                if hp == 3 and qcb + 1 < n_qc:
                    emit_mask_dma(qcb + 1, range(0, 1))
                for pair in range(n_kt // 2):
                    # force any overdue projection prerequisites
                    while F1 and F1[0][1] <= (ui, pair):
                        emit_f1_item(F1.pop(0)[0])
                    force_drain(6)
                    emit_scores_pair(qcb, hp, pair)
                    while est["pe"] < est["act"]:
                        if not pick_and_emit(ui):
                            break

            # drain remaining attn@V / norms / fillers
            while av_i[0] < len(av_steps):
                step = av_steps[av_i[0]]
                if (step[0] == "pair" and vp_next[0] <= step[3]):
                    ensure_xv(vp_next[0] // 2)
                    emit_vp_chunk(vp_next[0])
                    vp_next[0] += 1
                    continue
                emit_attnv_step(step)
            while F3:
                qcb, dm = F3.pop(0)
                emit_outproj_dm(qcb, dm)

            # final qcb out-proj: rotate through the idle cps ring for
            # pipelined evacuation (attention is done).
            qcb = n_qc - 1

            def close_dm(dm, op):
                nc.tensor.matmul(
                    op[:, :],
                    wo_tiles[n_do - 1][:, dm * 128:(dm + 1) * 128],
                    ctxT[n_do - 1][:, qcb * qc:(qcb + 1) * qc],
                    start=False, stop=True,
                )
                osb = osb_pool.tile([128, 512], f16, tag="osb", name="osb")
                nc.vector.tensor_copy(osb[:, :], op[:, :])
                nc.sync.dma_start(
                    oT[dm * 128:(dm + 1) * 128, qcb * qc:(qcb + 1) * qc],
                    osb[:, :],
                )

            pend = []
            for dm in range(n_di):
                if len(pend) == 4:
                    close_dm(*pend.pop(0))
                op = (o_pool if dm == 0 else c_pool).tile(
                    [128, 512], f32, tag="o" if dm == 0 else "c", name="ops")
                for t in range(n_do - 1):
                    nc.tensor.matmul(
                        op[:, :],
                        wo_tiles[t][:, dm * 128:(dm + 1) * 128],
                        ctxT[t][:, qcb * qc:(qcb + 1) * qc],
                        start=(t == 0), stop=False,
                    )
                pend.append((dm, op))
            for dm, op in pend:
                close_dm(dm, op)

    nc.compile()
    return nc


def make_in_maps(Q, K, V, mask, Wq, bq, Wk, bk, Wv, bv, Wo):
    Q = np.asarray(Q, np.float32)
    K = np.asarray(K, np.float32)
    V = np.asarray(V, np.float32)
    mask = np.asarray(mask)
    n_do = DH // 128
    in_maps = []
    for c in range(N_CORES):
        b, hg = c // HG, c % HG
        cs = slice(hg * DH, (hg + 1) * DH)
        in_maps.append({
            "xqT": np.ascontiguousarray(Q[b].T).astype(np.float16),
            "xkT": np.ascontiguousarray(K[b].T).astype(np.float16),
            "xvT": np.ascontiguousarray(V[b].T).astype(np.float16),
            "maskT": np.ascontiguousarray(mask[b, 0].T).astype(np.float16),
            "wq": np.asarray(Wq, np.float32)[:, cs].astype(np.float16),
            "wk": np.asarray(Wk, np.float32)[:, cs].astype(np.float16),
            "wv": np.asarray(Wv, np.float32)[:, cs].astype(np.float16),
            "wo": np.asarray(Wo, np.float32)[cs, :].astype(np.float16),
            "bqT": np.ascontiguousarray(
                np.asarray(bq, np.float32)[cs].reshape(n_do, 128).T),
            "bkT": np.ascontiguousarray(
                np.asarray(bk, np.float32)[cs].reshape(n_do, 128).T),
            "bv": np.asarray(bv, np.float32)[cs].reshape(1, DH)
                .astype(ml_dtypes.bfloat16),
            "ones_d": np.ones((1, 512), ml_dtypes.bfloat16),
        })
    return in_maps


def combine_outputs(results, bo):
    out = np.empty((B, S, D), np.float32)
    for b in range(B):
        out[b] = (results[HG * b]["oT"].astype(np.float32)
                  + results[HG * b + 1]["oT"].astype(np.float32)).T
    out += np.asarray(bo, np.float32)
    return out


def kernel(Q, K, V, mask, Wq, bq, Wk, bk, Wv, bv, Wo, bo):
    from concourse.bass_utils import run_bass_kernel_spmd

    in_maps = make_in_maps(Q, K, V, mask, Wq, bq, Wk, bk, Wv, bv, Wo)
    nc = build_attention_nc()
    res = run_bass_kernel_spmd(nc, in_maps, core_ids=list(range(N_CORES)))
    return combine_outputs(res.results, bo)
